# revision 3
# baseline (speedup 1.0000x reference)
"""GroupedQueryAttention Trainium2 kernel v3 (8 NeuronCores).

Problem: B=4, S=2048 queries, N=2048 keys, d_model=2048, G=16 heads,
d_head=128, RoPE (rotary_dim=512) on query only, key mask,
out = (softmax(mask(QK^T/sqrt(dh))) @ V) @ Wo^T.

Sharding: mesh = 4 batches x 2 head-halves. core_id = b*2 + h.
Each core: projections for its batch/head-half, attention for its 8 heads,
pair AllGather of context^T per query chunk, O-projection of its 1024
output columns, pipelined so the collectives hide under attention compute.

v3 over v2:
  - query chunks [512,512,512,256,256]: the two small tail chunks shrink
    the last (exposed) AllGather.
  - u accumulator copied PSUM->SBUF right after its last matmul, freeing
    the single-buffered u PSUM slot for the next head.
  - phase-K inputs DMA'd per k-tile so the first matmuls start earlier.
"""
import sys
import numpy as np

sys.path.insert(0, "/opt/trn_rl_repo")

from contextlib import ExitStack

import concourse.bass as bass
import concourse.tile as tile
from concourse import bacc, mybir
from concourse.bass_utils import run_bass_kernel_spmd
from concourse.tile import add_dep_helper

FP32 = mybir.dt.float32
BF16 = mybir.dt.bfloat16

B = 4
S = 2048          # queries per batch
D = 2048          # d_model
G = 16            # heads
DH = 128          # head dim
RD = 512          # rotary dim
TP = 2            # head-half split
CL = D // TP      # local channels (1024)
GL = G // TP      # local heads (8)
OC = D // TP      # output cols per core (1024)
SCALE = 1.0 / float(np.sqrt(DH))

KT = D // 128     # contraction k-tiles (16)
SC = S // 512     # 512-query chunks for the Q projection (4)
CT = D // 128     # context c-tiles (16)

QCHUNKS = [(0, 256), (256, 512), (768, 512), (1280, 512), (1792, 256)]


assert all(w % 128 == 0 for _, w in QCHUNKS) and sum(w for _, w in QCHUNKS) == S


def _chunks(total, step):
    out = []
    off = 0
    while off < total:
        w = min(step, total - off)
        out.append((off, w))
        off += w
    return out


def _build_program(NT):
    NK = NT * 128
    nc = bacc.Bacc("TRN2", target_bir_lowering=False, debug=False, num_devices=8)

    # ---- external I/O (per-core contents differ; same shapes) ----
    xq = nc.dram_tensor("xq", [D, S], BF16, kind="ExternalInput").ap()     # query^T
    xk = nc.dram_tensor("xk", [D, NK], BF16, kind="ExternalInput").ap()    # compacted key^T
    xv = nc.dram_tensor("xv", [D, NK], BF16, kind="ExternalInput").ap()    # compacted value^T
    wq = nc.dram_tensor("wq", [D, CL], BF16, kind="ExternalInput").ap()    # Wq[hs,:]^T
    wk = nc.dram_tensor("wk", [D, CL], BF16, kind="ExternalInput").ap()
    wv = nc.dram_tensor("wv", [D, CL], BF16, kind="ExternalInput").ap()
    wo = nc.dram_tensor("wo", [D, OC], BF16, kind="ExternalInput").ap()    # Wo^T[:, ocs]
    cosT = nc.dram_tensor("cosT", [RD, S], FP32, kind="ExternalInput").ap()
    sinT = nc.dram_tensor("sinT", [RD, S], FP32, kind="ExternalInput").ap()  # signed
    padfix = nc.dram_tensor("padfix", [128, 512], BF16, kind="ExternalInput").ap()
    ones_c = nc.dram_tensor("ones_c", [128, 1], BF16, kind="ExternalInput").ap()
    ones_r = nc.dram_tensor("ones_r", [1, 128], BF16, kind="ExternalInput").ap()
    out = nc.dram_tensor("out", [S, OC], FP32, kind="ExternalOutput").ap()

    # ---- DRAM scratch: context^T per query chunk (contiguous for AG) ----
    ct_local = [
        nc.dram_tensor(f"ct_local{i}", [CL, w], BF16).ap()
        for i, (off, w) in enumerate(QCHUNKS)
    ]
    ct_gath = [
        nc.dram_tensor(f"ct_gath{i}", [D, w], BF16).ap()
        for i, (off, w) in enumerate(QCHUNKS)
    ]

    xq_r = xq.rearrange("(kt p) s -> p kt s", p=128)
    xk_r = xk.rearrange("(kt p) n -> p kt n", p=128)
    xv_r = xv.rearrange("(kt p) n -> p kt n", p=128)
    cos_r = cosT.rearrange("(gt p) s -> p gt s", p=128)
    sin_r = sinT.rearrange("(gt p) s -> p gt s", p=128)

    kchunks = _chunks(NK, 512)

    with tile.TileContext(nc) as tc:
        with ExitStack() as top:
            consts = top.enter_context(tc.tile_pool(name="consts", bufs=1))
            pad_t = consts.tile([128, 512], BF16)
            ones_ct = consts.tile([128, 1], BF16)
            ones_rt = consts.tile([1, 128], BF16)
            nc.sync.dma_start(out=pad_t, in_=padfix)
            nc.sync.dma_start(out=ones_ct, in_=ones_c)
            nc.sync.dma_start(out=ones_rt, in_=ones_r)

            # resident activations
            kq_pool = top.enter_context(tc.tile_pool(name="kqres", bufs=1))
            k_sb = kq_pool.tile([128, GL, NK], BF16)    # K^T per head (dh x keys)
            v_sb = kq_pool.tile([128, NT, CL], BF16)    # V natural (keys x ch)
            q_sb = kq_pool.tile([128, GL, S], BF16)     # Q^T per head (dh x queries)

            # staggered-lifetime input pools (manually released to free SBUF):
            # wk/xk live for phase K only; wv/xv through phase V; wq through
            # phase Q; wo until the end. DMAs are issued one phase ahead.
            wkp = tc.alloc_tile_pool(name="wkpool", bufs=1, side="left")
            wvp = tc.alloc_tile_pool(name="wvpool", bufs=1, side="right")

            wk_t = wkp.tile([128, KT, CL], BF16)
            xk_t = wkp.tile([128, KT, NK], BF16)
            wk_r = wk.rearrange("(kt p) c -> p kt c", p=128)
            for kt in range(KT):
                nc.sync.dma_start(out=wk_t[:, kt, :], in_=wk_r[:, kt, :])
                nc.sync.dma_start(out=xk_t[:, kt, :], in_=xk_r[:, kt, :])
            wv_t = wvp.tile([128, KT, CL], BF16)
            xv_t = wvp.tile([128, KT, NK], BF16)
            nc.sync.dma_start(out=wv_t, in_=wv.rearrange("(kt p) c -> p kt c", p=128))
            nc.sync.dma_start(out=xv_t, in_=xv_r)

            # ---------- Phase K: K-projection -> k_sb ----------
            with ExitStack() as ph:
                pps = ph.enter_context(tc.tile_pool(name="kps", bufs=2, space="PSUM"))
                for g in range(GL):
                    ps = [
                        pps.tile([128, w], FP32, name=f"kp{ci}", tag=f"kp{ci}")
                        for ci, (off, w) in enumerate(kchunks)
                    ]
                    for kt in range(KT):
                        for ci, (off, w) in enumerate(kchunks):
                            nc.tensor.matmul(
                                out=ps[ci],
                                lhsT=wk_t[:, kt, g * 128:(g + 1) * 128],
                                rhs=xk_t[:, kt, off:off + w],
                                start=(kt == 0),
                                stop=(kt == KT - 1),
                            )
                    for ci, (off, w) in enumerate(kchunks):
                        nc.vector.tensor_copy(out=k_sb[:, g, off:off + w], in_=ps[ci])
            wkp.release()

            # prefetch Q weights during phase V
            wqp = tc.alloc_tile_pool(name="wqpool", bufs=1, side="left")
            wq_t = wqp.tile([128, KT, CL], BF16)
            nc.sync.dma_start(out=wq_t, in_=wq.rearrange("(kt p) c -> p kt c", p=128))

            # ---------- Phase V: V-projection -> v_sb (natural) ----------
            with ExitStack() as ph:
                pps = ph.enter_context(tc.tile_pool(name="vps", bufs=2, space="PSUM"))
                for nt in range(NT):
                    ps = [
                        pps.tile([128, 512], FP32, name=f"vp{cc}", tag=f"vp{cc}")
                        for cc in range(2)
                    ]
                    for kt in range(KT):
                        for cc in range(2):
                            nc.tensor.matmul(
                                out=ps[cc],
                                lhsT=xv_t[:, kt, nt * 128:(nt + 1) * 128],
                                rhs=wv_t[:, kt, cc * 512:(cc + 1) * 512],
                                start=(kt == 0),
                                stop=(kt == KT - 1),
                            )
                    for cc in range(2):
                        nc.vector.tensor_copy(
                            out=v_sb[:, nt, cc * 512:(cc + 1) * 512], in_=ps[cc]
                        )
            wvp.release()

            # prefetch O weights during phase Q
            wop = tc.alloc_tile_pool(name="wopool", bufs=1, side="right")
            wo_t = wop.tile([128, CT, OC], BF16)
            nc.sync.dma_start(out=wo_t, in_=wo.rearrange("(ct p) c -> p ct c", p=128))

            # ---------- Phase Q: Q-projection + RoPE -> q_sb ----------
            with ExitStack() as ph:
                xpool = ph.enter_context(tc.tile_pool(name="xqpool", bufs=2))
                rpool = ph.enter_context(tc.tile_pool(name="ropepool", bufs=2))
                rsc = ph.enter_context(tc.tile_pool(name="ropescratch", bufs=2))
                pps = ph.enter_context(tc.tile_pool(name="qps", bufs=1, space="PSUM"))

                for sc in range(SC):
                    ssl = slice(sc * 512, (sc + 1) * 512)
                    xq_t = xpool.tile([128, KT, 512], BF16, name="xq_t", tag="xq")
                    nc.sync.dma_start(out=xq_t, in_=xq_r[:, :, ssl])
                    cos_t = rpool.tile([128, 4, 512], FP32, name="cos_t", tag="cos")
                    sin_t = rpool.tile([128, 4, 512], FP32, name="sin_t", tag="sin")
                    nc.sync.dma_start(out=cos_t, in_=cos_r[:, :, ssl])
                    nc.sync.dma_start(out=sin_t, in_=sin_r[:, :, ssl])

                    ps = [
                        pps.tile([128, 512], FP32, name=f"qp{g}", tag=f"qp{g}")
                        for g in range(GL)
                    ]
                    for kt in range(KT):
                        for g in range(GL):
                            nc.tensor.matmul(
                                out=ps[g],
                                lhsT=wq_t[:, kt, g * 128:(g + 1) * 128],
                                rhs=xq_t[:, kt, :],
                                start=(kt == 0),
                                stop=(kt == KT - 1),
                            )
                    # heads 4..7: plain copy; heads 0..3: rope from pair g^2
                    for g in range(4, GL):
                        nc.vector.tensor_copy(out=q_sb[:, g, ssl], in_=ps[g])
                    for g in range(4):
                        sA = rsc.tile([128, 512], FP32, name="ropeA", tag="ropeA")
                        sB = rsc.tile([128, 512], FP32, name="ropeB", tag="ropeB")
                        nc.vector.tensor_mul(out=sA, in0=ps[g], in1=cos_t[:, g, :])
                        nc.vector.tensor_mul(out=sB, in0=ps[g ^ 2], in1=sin_t[:, g, :])
                        nc.vector.tensor_add(out=q_sb[:, g, ssl], in0=sA, in1=sB)

            # ---------- Attention + AllGather + O-projection, per query chunk ----
            with ExitStack() as ph:
                epool = ph.enter_context(tc.tile_pool(name="epool", bufs=6))
                uspool = ph.enter_context(tc.tile_pool(name="uspool", bufs=2))
                cpool = ph.enter_context(tc.tile_pool(name="cpool", bufs=2))
                bpool = ph.enter_context(tc.tile_pool(name="bpool", bufs=2))
                rpool = ph.enter_context(tc.tile_pool(name="rpool", bufs=2))
                copool = ph.enter_context(tc.tile_pool(name="copool", bufs=3))
                oopool = ph.enter_context(tc.tile_pool(name="oopool", bufs=2))
                sps = ph.enter_context(tc.tile_pool(name="sps", bufs=2, space="PSUM"))
                ups = ph.enter_context(tc.tile_pool(name="ups", bufs=1, space="PSUM"))
                dps = ph.enter_context(tc.tile_pool(name="dps", bufs=1, space="PSUM"))
                ops = ph.enter_context(tc.tile_pool(name="ops", bufs=1, space="PSUM"))

                NQ = len(QCHUNKS)
                last_attn_mm = [None] * NQ

                def o_proj(ci, after_mm):
                    # O-projection for one gathered query chunk.
                    # Everything downstream of the AllGather is kept off SP's
                    # and DVE's in-order queues until the next attention chunk
                    # is emitted there (Pool DMAs + explicit ordering deps),
                    # else the collective wait would stall the whole pipeline.
                    qoff, w = QCHUNKS[ci]
                    ct_r = ct_gath[ci].rearrange("(ct p) s -> p ct s", p=128)
                    first_copy_dep = after_mm
                    dep_cc = [after_mm, after_mm]
                    for st in range(w // 128):
                        c_sb = copool.tile([128, CT, 128], BF16, name="c_sb", tag="c_sb")
                        nc.gpsimd.dma_start(
                            out=c_sb, in_=ct_r[:, :, st * 128:(st + 1) * 128]
                        )
                        pso = [
                            ops.tile([128, 512], FP32, name=f"op{cc}", tag=f"op{cc}")
                            for cc in range(2)
                        ]
                        for ct in range(CT):
                            for cc in range(2):
                                mm = nc.tensor.matmul(
                                    out=pso[cc],
                                    lhsT=c_sb[:, ct, :],
                                    rhs=wo_t[:, ct, cc * 512:(cc + 1) * 512],
                                    start=(ct == 0),
                                    stop=(ct == CT - 1),
                                )
                                if dep_cc[cc] is not None:
                                    # keep O's PE work (both psum chains)
                                    # behind the next attention chunk so the
                                    # AllGather hides under it
                                    add_dep_helper(
                                        mm.ins, dep_cc[cc].ins, sync=True,
                                        reason="o_proj after next attn chunk",
                                    )
                                    dep_cc[cc] = None
                        o_sb = oopool.tile([128, OC], FP32, name="o_sb", tag="o_sb")
                        for cc in range(2):
                            cp = nc.vector.tensor_copy(
                                out=o_sb[:, cc * 512:(cc + 1) * 512], in_=pso[cc]
                            )
                            if first_copy_dep is not None:
                                add_dep_helper(
                                    cp.ins, first_copy_dep.ins, sync=True,
                                    reason="o_proj copies after next attn chunk",
                                )
                                first_copy_dep = None
                        nc.gpsimd.dma_start(
                            out=out[qoff + st * 128:qoff + (st + 1) * 128, :],
                            in_=o_sb,
                        )

                for ci, (qoff, w) in enumerate(QCHUNKS):
                    qsl = slice(qoff, qoff + w)
                    # pack as many key-tiles per s-psum/exp as fit in 1024 cols
                    P = max(1, 1024 // w)
                    ntgroups = [
                        tuple(range(i, min(i + P, NT))) for i in range(0, NT, P)
                    ]
                    for g in range(GL):
                        u_ps = ups.tile([128, 512], FP32, name="u_ps", tag="u")
                        d_ps = dps.tile([128, 512], FP32, name="d_ps", tag="d")
                        for pair in ntgroups:
                            s_ps = sps.tile([128, 1024], FP32, name="s_ps", tag="s")
                            for j, nt in enumerate(pair):
                                nc.tensor.matmul(
                                    out=s_ps[:, j * w:(j + 1) * w],
                                    lhsT=k_sb[:, g, nt * 128:(nt + 1) * 128],
                                    rhs=q_sb[:, g, qsl],
                                    start=True,
                                    stop=True,
                                )
                            e_t = epool.tile([128, 1024], BF16, name="e_t", tag="e")
                            we = w * len(pair)
                            nc.scalar.activation(
                                out=e_t[:, :we], in_=s_ps[:, :we],
                                func=mybir.ActivationFunctionType.Exp,
                                scale=SCALE,
                            )
                            for j, nt in enumerate(pair):
                                esl = slice(j * w, (j + 1) * w)
                                nc.tensor.matmul(
                                    out=u_ps[:, :w],
                                    lhsT=v_sb[:, nt, g * 128:(g + 1) * 128],
                                    rhs=e_t[:, esl],
                                    start=(nt == 0),
                                    stop=(nt == NT - 1),
                                )
                                nc.tensor.matmul(
                                    out=d_ps[0:1, :w],
                                    lhsT=ones_ct,
                                    rhs=e_t[:, esl],
                                    start=(nt == 0),
                                    stop=False,
                                )
                        # exact pad correction: sum_p pad_t[p, q] == -n_pad
                        nc.tensor.matmul(
                            out=d_ps[0:1, :w], lhsT=ones_ct, rhs=pad_t[:, :w],
                            start=False, stop=True,
                        )
                        # free the u PSUM slot right away for the next head
                        u_sb = uspool.tile([128, 512], FP32, name="u_sb", tag="usb")
                        nc.vector.tensor_copy(out=u_sb[:, :w], in_=u_ps[:, :w])
                        r_t = rpool.tile([1, 512], BF16, name="r_t", tag="r")
                        with nc.allow_low_precision(reason="softmax scale"):
                            nc.vector.reciprocal(out=r_t[:, :w], in_=d_ps[0:1, :w])
                        # broadcast 1/d over 128 partitions via matmul, reusing d tile
                        bmm = nc.tensor.matmul(
                            out=d_ps[:, :w], lhsT=ones_rt, rhs=r_t[:, :w],
                            start=True, stop=True,
                        )
                        if g == GL - 1:
                            last_attn_mm[ci] = bmm
                        b_sb = bpool.tile([128, 512], FP32, name="b_sb", tag="b")
                        nc.vector.tensor_copy(out=b_sb[:, :w], in_=d_ps[:, :w])
                        c_t = cpool.tile([128, 512], BF16, name="c_t", tag="c")
                        nc.vector.tensor_mul(
                            out=c_t[:, :w], in0=u_sb[:, :w], in1=b_sb[:, :w]
                        )
                        nc.sync.dma_start(
                            out=ct_local[ci][g * 128:(g + 1) * 128, :],
                            in_=c_t[:, :w],
                        )

                    # AllGather context^T for this query chunk within batch pair
                    nc.gpsimd.collective_compute(
                        "AllGather",
                        mybir.AluOpType.bypass,
                        replica_groups=[[0, 1], [2, 3], [4, 5], [6, 7]],
                        ins=[ct_local[ci]],
                        outs=[ct_gath[ci]],
                    )
                    # O-projection lags one chunk so AG(ci) hides under
                    # attention compute of chunk ci+1.
                    if ci >= 1:
                        o_proj(ci - 1, last_attn_mm[ci])
                o_proj(NQ - 1, None)
            wqp.release()
            wop.release()

    nc.compile()
    return nc


_NC_CACHE = {}


def _get_program(NT=9):
    if NT not in _NC_CACHE:
        _NC_CACHE[NT] = _build_program(NT)
    return _NC_CACHE[NT]


LAST_RESULTS = None


def kernel(query, key, value, mask, position_ids, Wq, Wk, Wv, Wo, **kw):
    import ml_dtypes

    bf16 = ml_dtypes.bfloat16
    query = np.asarray(query, dtype=np.float32)
    key = np.asarray(key, dtype=np.float32)
    value = np.asarray(value, dtype=np.float32)
    mask = np.asarray(mask)
    position_ids = np.asarray(position_ids)
    Wq = np.asarray(Wq, dtype=np.float32)
    Wk = np.asarray(Wk, dtype=np.float32)
    Wv = np.asarray(Wv, dtype=np.float32)
    Wo = np.asarray(Wo, dtype=np.float32)

    # ---- compact masked keys on host (exact: masked keys get 0 weight) ----
    idxs = [np.nonzero(mask[b] != 0)[0] for b in range(B)]
    nk_max = max(1, max(len(ix) for ix in idxs))
    NT = (nk_max + 127) // 128
    NK = NT * 128

    # rope tables from actual position_ids (applied to query only)
    pos = position_ids.astype(np.float64)
    freq = np.arange(0, RD, 2, dtype=np.float64)
    inv_freq = 1.0 / (10000.0 ** (freq / RD))
    pe = pos[:, None] * inv_freq[None, :]
    cos_half = np.cos(pe).astype(np.float32)
    sin_half = np.sin(pe).astype(np.float32)
    cosT_full = np.tile(cos_half, (1, 2)).T.copy()       # (512, S)
    sin_full = np.tile(sin_half, (1, 2)).T
    sinT_signed = sin_full.copy()
    sinT_signed[: RD // 2] *= -1.0                       # partner sign
    cosT_id = np.ones((RD, S), np.float32)
    sinT_id = np.zeros((RD, S), np.float32)

    ones_c = np.ones((128, 1), bf16)
    ones_r = np.ones((1, 128), bf16)

    in_maps = []
    for core in range(8):
        b, h = core // 2, core % 2
        hs = slice(h * CL, (h + 1) * CL)
        ix = idxs[b]
        nk_b = len(ix)
        n_pad = NK - nk_b
        xk_c = np.zeros((D, NK), bf16)
        xv_c = np.zeros((D, NK), bf16)
        xk_c[:, :nk_b] = key[b][ix].T.astype(bf16)
        xv_c[:, :nk_b] = value[b][ix].T.astype(bf16)
        # integer per-partition correction summing exactly to -n_pad
        pf = np.full(128, -(n_pad // 128), np.float32)
        pf[: n_pad % 128] -= 1.0
        padfix = np.broadcast_to(pf[:, None], (128, 512))
        in_maps.append({
            "xq": np.ascontiguousarray(query[b].T).astype(bf16),
            "xk": xk_c,
            "xv": xv_c,
            "wq": np.ascontiguousarray(Wq[hs, :].T).astype(bf16),
            "wk": np.ascontiguousarray(Wk[hs, :].T).astype(bf16),
            "wv": np.ascontiguousarray(Wv[hs, :].T).astype(bf16),
            "wo": np.ascontiguousarray(Wo.T[:, hs]).astype(bf16),
            "cosT": cosT_full if h == 0 else cosT_id,
            "sinT": sinT_signed if h == 0 else sinT_id,
            "padfix": np.ascontiguousarray(padfix).astype(bf16),
            "ones_c": ones_c,
            "ones_r": ones_r,
        })

    nc = _get_program(NT)
    res = run_bass_kernel_spmd(nc, in_maps, core_ids=list(range(8)))
    global LAST_RESULTS
    LAST_RESULTS = res

    out = np.empty((B, S, D), np.float32)
    for core in range(8):
        b, h = core // 2, core % 2
        out[b][:, h * OC:(h + 1) * OC] = res.results[core]["out"]
    return out
